# revision 2
# baseline (speedup 1.0000x reference)
"""AdaptiveWindowAttention distributed Bass kernel for 8 TRN2 NeuronCores.

Sharding: core c -> batch b = c//4, head group g = c%4 (heads 4g..4g+3).
 - QKV projection: bf16 matmuls (x, weights shipped pre-transposed+bf16),
   f32 PSUM accumulation. wq shipped host-packed so each mt group is one
   contiguous 4KB-per-partition DMA.
 - RoPE rotate-half via SBUF->SBUF DMA pair-swap of adjacent partitions
   (negation folded into the host-built sin table) + DVE combine -> bf16 q/k.
 - Windowed causal attention: window <= 256 -> 384-wide band per q-tile
   against 256-zero-padded K/V (edge q-tiles trimmed). Mask via DVE add of
   host-built bias tables; exp+rowsum on Act; normalize on DVE; probs bf16.
 - P transposed on PE (bf16, 1 cyc/row), AV + out-projection feed from
   SBUF-resident bf16 tiles. Fully fused pipeline: score/softmax chains and
   transpose+AV are emitted interleaved inside the projection matmul streams
   so no engine queue head-of-line blocks.
 - Out-projection partials written bf16 per 128-row tile; per-tile
   ReduceScatter (add) across each 4-core group into an internal DRAM
   buffer, then copied to the bf16 output. All output-path DMA dispatches
   are lag-staggered so the in-order sync queue never waits. Host casts to
   f32 and reassembles the full [2,2048,2048] output.
"""
import os
import sys

import numpy as np

for _p in ("/opt/trn_rl_repo",):
    if _p not in sys.path and os.path.isdir(_p):
        sys.path.insert(0, _p)

EMB = 2048
HEADS = 16
HD = 128
S = 2048
B = 2
SCALE = HD ** -0.5
NCORES = 8

_CACHE = {}


# ----------------------------------------------------------------- host math
def _host_window(x, w_c1, w_c2):
    xf = x.reshape(B, -1).astype(np.float64)
    var = xf.var(axis=1, ddof=1)
    var_norm = 1.0 / (1.0 + np.exp(-(var * 10.0 - 5.0)))
    x_mean = x.mean(axis=1).astype(np.float64)
    h = x_mean @ w_c1.T.astype(np.float64)
    h = h / (1.0 + np.exp(-h))
    learned = 1.0 / (1.0 + np.exp(-(h @ w_c2.T.astype(np.float64))))[:, 0]
    complexity = (var_norm + learned) / 2.0
    window_f = 64.0 + complexity * (256.0 - 64.0)
    w = int(np.float32(window_f.mean()))
    return max(min(w, S), 64)


def _build_maskbias(window):
    NEG = -1.0e5
    mb = np.empty((128, 3, 384), dtype=np.float32)
    jmin = [256, 128, 0]
    q = np.arange(128)[:, None]
    j = np.arange(384)[None, :]
    rel = q + 256 - j
    for v in range(3):
        keep = (rel >= 0) & (rel < window) & (j >= jmin[v])
        mb[:, v, :] = np.where(keep, 0.0, NEG)
    return mb


def _rope_tables():
    inv_freq = 1.0 / (10000.0 ** (np.arange(0, HD, 2, dtype=np.float32) / HD))
    pos = np.arange(S, dtype=np.float32)
    freqs = np.outer(pos, inv_freq)
    emb = np.concatenate([freqs, freqs], axis=-1)  # [S, 128]
    cosT = np.ascontiguousarray(np.cos(emb).T).astype(np.float32)
    sinT = np.ascontiguousarray(np.sin(emb).T).astype(np.float32)
    # negation of rotate-half folded into the sin table: even rows flip sign
    # (dest = pairswap(q) * sinT_alt, pairswap has no negation)
    sinT_alt = sinT.copy()
    sinT_alt[0::2, :] *= -1.0
    return cosT, sinT_alt


# ----------------------------------------------------------------- bass build
def _build_nc(single_core=False, phases=3, iters=1):
    import concourse.bass as bass  # noqa: F401
    from concourse import bacc, mybir, tile

    f32 = mybir.dt.float32
    bf16 = mybir.dt.bfloat16
    AF = mybir.ActivationFunctionType
    ALU = mybir.AluOpType

    nc = bacc.Bacc("TRN2", target_bir_lowering=False, debug=False,
                   num_devices=1 if single_core else NCORES)

    xT_d = nc.dram_tensor("xT", [EMB, S], bf16, kind="ExternalInput").ap()
    wqkP_d = nc.dram_tensor("wqkP", [8 * 128, S], bf16, kind="ExternalInput").ap()
    wvT_d = nc.dram_tensor("wvT", [EMB, 512], bf16, kind="ExternalInput").ap()
    woT_d = nc.dram_tensor("woT", [512, EMB], bf16, kind="ExternalInput").ap()
    cosT_d = nc.dram_tensor("cosT", [HD, S], bf16, kind="ExternalInput").ap()
    sinT_d = nc.dram_tensor("sinT", [HD, S], bf16, kind="ExternalInput").ap()
    ident_d = nc.dram_tensor("ident", [HD, HD], bf16, kind="ExternalInput").ap()
    mb_d = nc.dram_tensor("mb", [128, 3, 384], f32, kind="ExternalInput").ap()
    out_d = nc.dram_tensor("out", [512, EMB], bf16, kind="ExternalOutput").ap()

    xT_r = xT_d.rearrange("(c p) s -> p c s", p=128)        # [128,16,S]
    wqkP_r = wqkP_d.rearrange("(m p) n -> p m n", p=128)    # [128,8,2048]
    wvT_r = wvT_d.rearrange("(c p) m -> p c m", p=128)      # [128,16,512]
    woT_r = woT_d.rearrange("(h p) n -> p h n", p=128)      # [128,4,EMB]

    RG = [[0, 1, 2, 3], [4, 5, 6, 7]]

    with tile.TileContext(nc) as tc:
        from contextlib import ExitStack
        with ExitStack() as ctx:
            resid = ctx.enter_context(tc.tile_pool(name="resid", bufs=1))
            dramp = ctx.enter_context(tc.tile_pool(name="dram", bufs=1, space="DRAM"))
            sp = ctx.enter_context(tc.tile_pool(name="sp", bufs=4))
            pxp = ctx.enter_context(tc.tile_pool(name="pxp", bufs=18))
            smp = ctx.enter_context(tc.tile_pool(name="smp", bufs=24))
            ptp = ctx.enter_context(tc.tile_pool(name="ptp", bufs=4))
            qkp = ctx.enter_context(tc.tile_pool(name="qkp", bufs=2))

            vP = resid.tile([128, 18, 512], bf16, tag="vP")
            ident_sb = resid.tile([128, 128], bf16, tag="ident")
            cs_sb = resid.tile([128, 2, S], bf16, tag="cs")
            mb_sb = resid.tile([128, 3, 384], f32, tag="mb")
            wo_sb = resid.tile([128, 4, EMB], bf16, tag="wo")
            attnT = resid.tile([128, 4, S], bf16, tag="attnT")
            zr = resid.tile([128, 512], bf16, tag="zr")

            nc.vector.memset(zr[:], 0.0)
            # warm-up matmuls on zeros: start the PE p-state ramp during the
            # initial input-DMA window so real matmuls run at full clock
            warm_stack = ExitStack()
            warmp = warm_stack.enter_context(
                tc.tile_pool(name="warm", bufs=1, space="PSUM"))
            wps = warmp.tile([128, 512], f32, tag="w")
            for _w in range(7):
                nc.tensor.matmul(wps[:], zr[:, 0:128], zr[:],
                                 start=True, stop=True)
            warm_stack.close()
            for _c in range(2):
                nc.vector.tensor_copy(vP[:, _c, :], zr[:])

            part_dram = dramp.tile([S, EMB], bf16)
            rs_dram = dramp.tile([512, EMB], bf16)

            for _it in range(iters):
              pexps = {}
              qTs = {}
              kTs = {}
              psp = None  # created after v_proj's psum pool is released

              def chain(p, h, qi):
                  # one full score->softmax chain for q-tile qt = 2p+qi.
                  # edge tiles qt=0,1 only see keys in the last 128/256 band
                  # columns (the rest is zero padding) -> trim the work
                  qt = 2 * p + qi
                  lo = 256 if qt == 0 else (128 if qt == 1 else 0)
                  w = 384 - lo
                  sc_ps = psp.tile([128, 384], f32, tag="sc",
                                   name=f"sc_{p}_{h}_{qi}", bufs=2)
                  nc.tensor.matmul(
                      sc_ps[:, 0:w],
                      qTs[h][:, qt * 128:(qt + 1) * 128],
                      kTs[h][:, qt * 128 + lo: qt * 128 + 384],
                      start=True, stop=True)
                  v_idx = min(qt, 2)
                  scb = sp.tile([128, 384], bf16, tag="scb",
                                name=f"scb_{p}_{h}_{qi}")
                  nc.vector.tensor_add(out=scb[:, 0:w], in0=sc_ps[:, 0:w],
                                       in1=mb_sb[:, v_idx, lo:384])
                  pexp = pxp.tile([128, 384], bf16, tag="pexp",
                                  name=f"pexp_{p}_{h}_{qi}")
                  rs = smp.tile([128, 1], f32, tag="rs")
                  nc.scalar.activation(pexp[:, 0:w], scb[:, 0:w], AF.Exp,
                                       bias=0.0, scale=SCALE,
                                       accum_out=rs[:])
                  rr = smp.tile([128, 1], f32, tag="rr")
                  nc.vector.reciprocal(rr[:], rs[:])
                  nc.vector.tensor_scalar_mul(pexp[:, 0:w], pexp[:, 0:w],
                                              rr[:])
                  pexps[(p, h, qi)] = (pexp, lo)

              def trans_av(p, h):
                  # transpose pexp tiles for (p, h) and contract against V
                  PT = ptp.tile([128, 4, 256], bf16, tag="PT",
                                name=f"PT_{p}_{h}")
                  if p == 0:
                      # qi=0 fills cc2 block0; qi=1 fills cc2,cc3 block1
                      nc.vector.tensor_copy(PT[:, 3, 0:128], zr[:, 0:128])
                  else:
                      nc.vector.tensor_copy(PT[:, 0, 128:256], zr[:, 0:128])
                      nc.vector.tensor_copy(PT[:, 3, 0:128], zr[:, 0:128])
                  for qi in range(2):
                      pexp, lo = pexps.pop((p, h, qi))
                      nch = (384 - lo) // 128
                      pt3 = psp.tile([128, 384], bf16, tag="pt",
                                     name=f"pt3_{p}_{h}_{qi}", bufs=2)
                      for j in range(nch):
                          nc.tensor.transpose(
                              pt3[:, j * 128:(j + 1) * 128],
                              pexp[:, j * 128:(j + 1) * 128],
                              ident_sb[:])
                      # trimmed band block j is key chunk qi + lo//128 + j
                      c0 = qi + lo // 128
                      if qi == 0:
                          nc.scalar.copy(
                              PT[:, c0:c0 + nch, qi * 128:(qi + 1) * 128],
                              pt3[:, 0:nch * 128].rearrange(
                                  "p (c q) -> p c q", c=nch))
                      else:
                          nc.vector.tensor_copy(
                              PT[:, c0:c0 + nch, qi * 128:(qi + 1) * 128],
                              pt3[:, 0:nch * 128].rearrange(
                                  "p (c q) -> p c q", c=nch))
                  av = psp.tile([128, 256], f32, tag="av",
                                name=f"av_{p}_{h}", bufs=1)
                  cc0 = 2 if p == 0 else 0
                  for cc in range(cc0, 4):
                      nc.tensor.matmul(
                          av[:],
                          vP[:, 2 * p + cc, h * 128:(h + 1) * 128],
                          PT[:, cc, :],
                          start=(cc == cc0), stop=(cc == 3))
                  if h % 2 == 0:
                      nc.scalar.copy(
                          attnT[:, h, p * 256:(p + 1) * 256], av[:])
                  else:
                      nc.vector.tensor_copy(
                          attnT[:, h, p * 256:(p + 1) * 256], av[:])

              # ------------- fused qkv projection + rope + attention chains
              with ExitStack() as p1:
                xp = p1.enter_context(tc.tile_pool(name="xp", bufs=16))
                wqp = p1.enter_context(tc.tile_pool(name="wqp", bufs=3))
                tp = p1.enter_context(tc.tile_pool(name="tp", bufs=4))
                wv_stack = ExitStack()
                wvp = wv_stack.enter_context(tc.tile_pool(name="wvp", bufs=1))

                xts = []
                wv_t = wvp.tile([128, 16, 512], bf16, tag="wv")
                for c in range(16):
                    xt = xp.tile([128, S], bf16, tag="xt", name=f"xt{c}")
                    nc.sync.dma_start(wv_t[:, c, :], wvT_r[:, c, :])
                    if c == 0:
                        nc.sync.dma_start(xt[:, 0:512], xT_r[:, c, 0:512])
                        nc.sync.dma_start(xt[:, 512:], xT_r[:, c, 512:])
                    else:
                        nc.sync.dma_start(xt[:], xT_r[:, c, :])
                    xts.append(xt)
                if _it == 0:
                    nc.sync.dma_start(ident_sb[:], ident_d[:])
                    nc.sync.dma_start(cs_sb[:, 0, :], cosT_d[:])
                    nc.sync.dma_start(cs_sb[:, 1, :], sinT_d[:])
                    nc.sync.dma_start(mb_sb[:], mb_d[:])
                    for h in range(4):
                        nc.sync.dma_start(wo_sb[:, h, :], woT_r[:, h, :])

                def v_proj(vpsp):
                    # V projection: direct [s, d] orientation (lhsT = x chunk)
                    for ss in range(16):
                        psv = vpsp.tile([128, 512], f32, tag="psv",
                                        name=f"psv_{ss}")
                        for c in range(16):
                            nc.tensor.matmul(
                                psv[:], xts[c][:, ss * 128:(ss + 1) * 128],
                                wv_t[:, c, :], start=(c == 0), stop=(c == 15))
                        nc.scalar.copy(vP[:, ss + 2, :], psv[:])

                vp_stack = ExitStack()
                vpsp = vp_stack.enter_context(
                    tc.tile_pool(name="vps", bufs=8, space="PSUM"))
                v_proj(vpsp)
                wv_stack.close()

                def mt_group(mt, fillers=(), every=4, pool=None):
                    fillers = list(fillers)
                    wq_t = wqp.tile([128, S], bf16, tag="wq")
                    nc.sync.dma_start(wq_t[:], wqkP_r[:, mt, :])
                    h = mt % 4
                    is_q = mt < 4
                    if is_q:
                        qTs[h] = qkp.tile([128, S], bf16, tag="qT",
                                          name=f"qT{h}")
                    else:
                        kTs[h] = qkp.tile([128, 256 + S], bf16, tag="kT",
                                          name=f"kT{h}")
                        nc.vector.tensor_copy(kTs[h][:, 0:256], zr[:, 0:256])
                    for n in range(4):
                        if pool is None:
                            ps = psp.tile([128, 512], f32, tag="ps",
                                          name=f"ps_{mt}_{n}")
                        else:
                            ps = pool.tile([128, 512], f32, tag="psv",
                                           name=f"ps_{mt}_{n}")
                        for c in range(16):
                            nc.tensor.matmul(
                                ps[:], wq_t[:, c * 128:(c + 1) * 128],
                                xts[c][:, n * 512:(n + 1) * 512],
                                start=(c == 0), stop=(c == 15))
                            if c % every == every - 1 and fillers:
                                fillers.pop(0)()
                        tmp = tp.tile([128, 512], bf16, tag="tmp",
                                      name=f"tmp_{mt}_{n}")
                        if n % 2 == 0:
                            nc.scalar.copy(tmp[:], ps[:])
                        else:
                            nc.vector.tensor_copy(tmp[:], ps[:])
                        # rotate-half via adjacent-partition pair swap (DMA);
                        # negation folded into sinT_alt
                        sw = tp.tile([128, 512], bf16, tag="sw",
                                     name=f"sw_{mt}_{n}", bufs=3)
                        tmp_r = tmp.rearrange("(g two) s -> g two s", two=2)
                        sw_r = sw.rearrange("(g two) s -> g two s", two=2)
                        nc.sync.dma_start(sw_r[:, 0, :], tmp_r[:, 1, :])
                        nc.sync.dma_start(sw_r[:, 1, :], tmp_r[:, 0, :])
                        if is_q:
                            dest = qTs[h][:, n * 512:(n + 1) * 512]
                        else:
                            dest = kTs[h][:, 256 + n * 512: 256 + (n + 1) * 512]
                        nc.vector.tensor_mul(
                            out=dest, in0=sw[:],
                            in1=cs_sb[:, 1, n * 512:(n + 1) * 512])
                        nc.vector.tensor_mul(
                            out=tmp[:], in0=tmp[:],
                            in1=cs_sb[:, 0, n * 512:(n + 1) * 512])
                        nc.vector.tensor_add(out=dest, in0=dest, in1=tmp[:])

                # ---- staggered out-projection emission
                obs = {}

                def out_mm(st):
                    sti = st % 2
                    ob = obp.tile([128, 2048], bf16, tag="ob",
                                  name=f"ob_{st}")
                    for nt in range(4):
                        po = psp.tile([128, 512], f32, tag="ps",
                                      name=f"po_{st}_{nt}")
                        for hh in range(4):
                            nc.tensor.matmul(
                                po[:], attnT[:, hh, st * 128:(st + 1) * 128],
                                wo_sb[:, hh, nt * 512:(nt + 1) * 512],
                                start=(hh == 0), stop=(hh == 3))
                        if (sti + nt) % 2 == 0:
                            nc.scalar.copy(
                                ob[:, nt * 512:(nt + 1) * 512], po[:])
                        else:
                            nc.vector.tensor_copy(
                                ob[:, nt * 512:(nt + 1) * 512], po[:])
                        if st == 15:
                            # last tile: per-quarter part writes so the final
                            # RS only waits on a 512-col DMA
                            nc.sync.dma_start(
                                part_dram[st * 128:(st + 1) * 128,
                                          nt * 512:(nt + 1) * 512],
                                ob[:, nt * 512:(nt + 1) * 512])
                    obs[st] = ob

                def out_part(st):
                    nc.sync.dma_start(
                        part_dram[st * 128:(st + 1) * 128, :], obs.pop(st))

                def out_rs(g0, G):
                    # ReduceScatter over sts [g0, g0+G): core j of the group
                    # receives G*32 contiguous rows at input offset
                    # g0*128 + j*G*32; stored at rs rows [g0*32, (g0+G)*32)
                    if phases < 3:
                        return
                    rows = G * 32
                    rsv = rs_dram.rearrange(
                        "s (b e) -> (s b) e",
                        b=4)[g0 * 128:g0 * 128 + rows * 4]
                    if single_core:
                        # surrogate: the j=0 chunk; flat views keep it wide
                        src = part_dram.rearrange(
                            "s (b e) -> (s b) e",
                            b=4)[g0 * 512:g0 * 512 + rows * 4]
                        nc.sync.dma_start(rsv, src)
                    else:
                        nc.gpsimd.collective_compute(
                            "ReduceScatter", ALU.add, replica_groups=RG,
                            ins=[part_dram[g0 * 128:(g0 + G) * 128, :].opt()],
                            outs=[rs_dram[g0 * 32:g0 * 32 + rows, :].opt()])

                def out_final(g0, G):
                    if phases < 3:
                        return
                    rows = G * 32
                    rsv = rs_dram.rearrange(
                        "s (b e) -> (s b) e",
                        b=4)[g0 * 128:g0 * 128 + rows * 4]
                    dst = out_d.rearrange(
                        "s (b e) -> (s b) e",
                        b=4)[g0 * 128:g0 * 128 + rows * 4]
                    nc.sync.dma_start(dst, rsv)

                def out_proj_st(st):
                    # part-DMA lags the matmuls so the in-order sync-queue
                    # dispatch never waits; grouped RS/out emitted on the
                    # group boundaries (one extra st of lag)
                    out_mm(st)
                    if st >= 1:
                        out_part(st - 1)
                    if st == 7:
                        out_rs(0, 6)
                    elif st == 8:
                        out_final(0, 6)
                    elif st == 13:
                        out_rs(6, 6)
                    elif st == 14:
                        out_final(6, 6)
                    elif st == 15:
                        out_rs(12, 2)
                        out_rs(14, 1)

                def out_proj_flush():
                    obs.pop(15)
                    out_final(12, 2)
                    out_final(14, 1)
                    out_rs(15, 1)
                    out_final(15, 1)

                # ---- fused emission schedule. q0 rides the v_proj psum
                # pool (no inter-pool zone dependency); the long-lived psum
                # pool is created after that pool retires.
                for h in range(4):
                    ch = []
                    if h >= 1:
                        for p in range(8):
                            for qi in range(2):
                                ch.append(lambda p=p, qi=qi, hh=h - 1:
                                          chain(p, hh, qi))
                    if h == 0:
                        mt_group(0, pool=vpsp)                # q_0
                        vp_stack.close()
                        psp = ctx.enter_context(
                            tc.tile_pool(name="ps", bufs=3, space="PSUM"))
                        obp = p1.enter_context(
                            tc.tile_pool(name="obp", bufs=4))
                    else:
                        mt_group(h, fillers=ch, every=4)      # q_h
                    ta = []
                    if h >= 1:
                        for p in range(8):
                            ta.append(lambda p=p, hh=h - 1: trans_av(p, hh))
                    if h == 3:
                        # also squeeze the first chains(3) into k3's stream;
                        # their kT3 bands are roped by the time these slots
                        # come up (n2/n3)
                        for p in (0, 1):
                            for qi in range(2):
                                ta.append(lambda p=p, qi=qi: chain(p, 3, qi))
                        mt_group(4 + h, fillers=ta, every=4)  # k_3
                    else:
                        mt_group(4 + h, fillers=ta, every=8)  # k_h

                # tail: chains(3) / trans_av(3) / out-projection pipelined
                for p in range(2, 8):
                    chain(p, 3, 0)
                    chain(p, 3, 1)
                    trans_av(p - 2, 3)
                    if p >= 3:
                        pp = p - 3
                        out_proj_st(2 * pp)
                        out_proj_st(2 * pp + 1)
                trans_av(6, 3)
                out_proj_st(10)
                out_proj_st(11)
                trans_av(7, 3)
                for st in (12, 13, 14, 15):
                    out_proj_st(st)
                out_proj_flush()

              if phases < 2:
                  ob1 = resid.tile([128, 512], f32, tag="ob1")
                  nc.vector.tensor_copy(ob1[:], qTs[3][:, 0:512])
                  nc.sync.dma_start(out_d[0:128, 0:512], ob1[:])
                  continue

    nc.compile()
    return nc


def _get_nc():
    if "nc" not in _CACHE:
        _CACHE["nc"] = _build_nc()
    return _CACHE["nc"]


# ----------------------------------------------------------------- entry
def _prepare_in_maps(x, w_qkv, w_out, w_c1, w_c2):
    import ml_dtypes

    x = np.asarray(x, dtype=np.float32)
    w_qkv = np.asarray(w_qkv, dtype=np.float32)
    w_out = np.asarray(w_out, dtype=np.float32)
    w_c1 = np.asarray(w_c1, dtype=np.float32)
    w_c2 = np.asarray(w_c2, dtype=np.float32)

    window = _host_window(x, w_c1, w_c2)
    mb = _build_maskbias(window)
    cosT, sinT = _rope_tables()
    ident = np.eye(128, dtype=np.float32)
    bf = ml_dtypes.bfloat16

    in_maps = []
    for c in range(NCORES):
        b, g = divmod(c, 4)
        rows = slice(g * 512, g * 512 + 512)
        wq = w_qkv[0 * EMB:1 * EMB][rows]
        wk = w_qkv[1 * EMB:2 * EMB][rows]
        wv = w_qkv[2 * EMB:3 * EMB][rows]
        # packed qk weights: wqkP[mt, p, c*128 + j] = wqk[e = c*128+p, mt*128+j]
        wqkT = np.concatenate([wq, wk], 0).T  # [EMB, 1024]
        wqkP = np.ascontiguousarray(
            wqkT.reshape(16, 128, 8, 128).transpose(2, 1, 0, 3)
        ).reshape(8 * 128, 2048)
        in_maps.append({
            "xT": np.ascontiguousarray(x[b].T).astype(bf),
            "wqkP": wqkP.astype(bf),
            "wvT": np.ascontiguousarray(wv.T).astype(bf),
            "woT": np.ascontiguousarray(w_out.T[rows]).astype(bf),
            "cosT": cosT.astype(bf), "sinT": sinT.astype(bf),
            "ident": ident.astype(bf),
            "mb": mb,
        })
    return in_maps


def _assemble(results):
    out = np.empty((B, S, EMB), dtype=np.float32)
    for c in range(NCORES):
        b, j = divmod(c, 4)
        r = np.asarray(results[c]["out"], dtype=np.float32)  # [512, EMB] bf16
        for g0, G in ((0, 6), (6, 6), (12, 2), (14, 1), (15, 1)):
            rows = G * 32
            out[b, g0 * 128 + j * rows: g0 * 128 + (j + 1) * rows] = \
                r[g0 * 32:g0 * 32 + rows]
    return out


def kernel(x, w_qkv, w_out, w_c1, w_c2):
    from concourse.bass_utils import run_bass_kernel_spmd

    nc = _get_nc()
    in_maps = _prepare_in_maps(x, w_qkv, w_out, w_c1, w_c2)
    res = run_bass_kernel_spmd(nc, in_maps, core_ids=list(range(NCORES)))
    return _assemble(res.results)


# revision 3
# speedup vs baseline: 1.0030x; 1.0030x over previous
"""AdaptiveWindowAttention distributed Bass kernel for 8 TRN2 NeuronCores.

Sharding: core c -> batch b = c//4, head group g = c%4 (heads 4g..4g+3).
 - QKV projection: bf16 matmuls (x, weights shipped pre-transposed+bf16),
   f32 PSUM accumulation. wq shipped host-packed so each mt group is one
   contiguous 4KB-per-partition DMA.
 - RoPE rotate-half via SBUF->SBUF DMA pair-swap of adjacent partitions
   (negation folded into the host-built sin table) + DVE combine -> bf16 q/k.
 - Windowed causal attention: window <= 256 -> 384-wide band per q-tile
   against 256-zero-padded K/V (edge q-tiles trimmed). Mask via DVE add of
   host-built bias tables; exp+rowsum on Act; normalize on DVE; probs bf16.
 - P transposed on PE (bf16, 1 cyc/row), AV + out-projection feed from
   SBUF-resident bf16 tiles. Fully fused pipeline: score/softmax chains and
   transpose+AV are emitted interleaved inside the projection matmul streams
   so no engine queue head-of-line blocks.
 - Out-projection partials written bf16 per 128-row tile; per-tile
   ReduceScatter (add) across each 4-core group into an internal DRAM
   buffer, then copied to the bf16 output. All output-path DMA dispatches
   are lag-staggered so the in-order sync queue never waits. Host casts to
   f32 and reassembles the full [2,2048,2048] output.
"""
import os
import sys

import numpy as np

for _p in ("/opt/trn_rl_repo",):
    if _p not in sys.path and os.path.isdir(_p):
        sys.path.insert(0, _p)

EMB = 2048
HEADS = 16
HD = 128
S = 2048
B = 2
SCALE = HD ** -0.5
NCORES = 8

_CACHE = {}


# ----------------------------------------------------------------- host math
def _host_window(x, w_c1, w_c2):
    xf = x.reshape(B, -1).astype(np.float64)
    var = xf.var(axis=1, ddof=1)
    var_norm = 1.0 / (1.0 + np.exp(-(var * 10.0 - 5.0)))
    x_mean = x.mean(axis=1).astype(np.float64)
    h = x_mean @ w_c1.T.astype(np.float64)
    h = h / (1.0 + np.exp(-h))
    learned = 1.0 / (1.0 + np.exp(-(h @ w_c2.T.astype(np.float64))))[:, 0]
    complexity = (var_norm + learned) / 2.0
    window_f = 64.0 + complexity * (256.0 - 64.0)
    w = int(np.float32(window_f.mean()))
    return max(min(w, S), 64)


def _build_maskbias(window):
    NEG = -1.0e5
    mb = np.empty((128, 3, 384), dtype=np.float32)
    jmin = [256, 128, 0]
    q = np.arange(128)[:, None]
    j = np.arange(384)[None, :]
    rel = q + 256 - j
    for v in range(3):
        keep = (rel >= 0) & (rel < window) & (j >= jmin[v])
        mb[:, v, :] = np.where(keep, 0.0, NEG)
    return mb


def _rope_tables():
    inv_freq = 1.0 / (10000.0 ** (np.arange(0, HD, 2, dtype=np.float32) / HD))
    pos = np.arange(S, dtype=np.float32)
    freqs = np.outer(pos, inv_freq)
    emb = np.concatenate([freqs, freqs], axis=-1)  # [S, 128]
    cosT = np.ascontiguousarray(np.cos(emb).T).astype(np.float32)
    sinT = np.ascontiguousarray(np.sin(emb).T).astype(np.float32)
    # negation of rotate-half folded into the sin table: even rows flip sign
    # (dest = pairswap(q) * sinT_alt, pairswap has no negation)
    sinT_alt = sinT.copy()
    sinT_alt[0::2, :] *= -1.0
    return cosT, sinT_alt


# ----------------------------------------------------------------- bass build
def _build_nc(single_core=False, phases=3, iters=1):
    import concourse.bass as bass  # noqa: F401
    from concourse import bacc, mybir, tile

    f32 = mybir.dt.float32
    bf16 = mybir.dt.bfloat16
    AF = mybir.ActivationFunctionType
    ALU = mybir.AluOpType

    nc = bacc.Bacc("TRN2", target_bir_lowering=False, debug=False,
                   num_devices=1 if single_core else NCORES)

    xT_d = nc.dram_tensor("xT", [EMB, S], bf16, kind="ExternalInput").ap()
    wqkP_d = nc.dram_tensor("wqkP", [8 * 128, S], bf16, kind="ExternalInput").ap()
    wvT_d = nc.dram_tensor("wvT", [EMB, 512], bf16, kind="ExternalInput").ap()
    woT_d = nc.dram_tensor("woT", [512, EMB], bf16, kind="ExternalInput").ap()
    cosT_d = nc.dram_tensor("cosT", [HD, S], bf16, kind="ExternalInput").ap()
    sinT_d = nc.dram_tensor("sinT", [HD, S], bf16, kind="ExternalInput").ap()
    ident_d = nc.dram_tensor("ident", [HD, HD], bf16, kind="ExternalInput").ap()
    mb_d = nc.dram_tensor("mb", [128, 3, 384], f32, kind="ExternalInput").ap()
    out_d = nc.dram_tensor("out", [512, EMB], bf16, kind="ExternalOutput").ap()

    xT_r = xT_d.rearrange("(c p) s -> p c s", p=128)        # [128,16,S]
    wqkP_r = wqkP_d.rearrange("(m p) n -> p m n", p=128)    # [128,8,2048]
    wvT_r = wvT_d.rearrange("(c p) m -> p c m", p=128)      # [128,16,512]
    woT_r = woT_d.rearrange("(h p) n -> p h n", p=128)      # [128,4,EMB]

    RG = [[0, 1, 2, 3], [4, 5, 6, 7]]

    with tile.TileContext(nc) as tc:
        from contextlib import ExitStack
        with ExitStack() as ctx:
            resid = ctx.enter_context(tc.tile_pool(name="resid", bufs=1))
            dramp = ctx.enter_context(tc.tile_pool(name="dram", bufs=1, space="DRAM"))
            sp = ctx.enter_context(tc.tile_pool(name="sp", bufs=4))
            pxp = ctx.enter_context(tc.tile_pool(name="pxp", bufs=18))
            smp = ctx.enter_context(tc.tile_pool(name="smp", bufs=24))
            ptp = ctx.enter_context(tc.tile_pool(name="ptp", bufs=4))
            qkp = ctx.enter_context(tc.tile_pool(name="qkp", bufs=2))

            vP = resid.tile([128, 18, 512], bf16, tag="vP")
            ident_sb = resid.tile([128, 128], bf16, tag="ident")
            cs_sb = resid.tile([128, 2, S], bf16, tag="cs")
            mb_sb = resid.tile([128, 3, 384], f32, tag="mb")
            wo_sb = resid.tile([128, 4, EMB], bf16, tag="wo")
            attnT = resid.tile([128, 4, S], bf16, tag="attnT")
            zr = resid.tile([128, 512], bf16, tag="zr")

            nc.vector.memset(zr[:], 0.0)
            # warm-up matmuls on zeros: start the PE p-state ramp during the
            # initial input-DMA window so real matmuls run at full clock
            warm_stack = ExitStack()
            warmp = warm_stack.enter_context(
                tc.tile_pool(name="warm", bufs=1, space="PSUM"))
            wps = warmp.tile([128, 512], f32, tag="w")
            for _w in range(7):
                nc.tensor.matmul(wps[:], zr[:, 0:128], zr[:],
                                 start=True, stop=True)
            warm_stack.close()
            for _c in range(2):
                nc.vector.tensor_copy(vP[:, _c, :], zr[:])

            part_dram = dramp.tile([S, EMB], bf16)
            rs_dram = dramp.tile([512, EMB], bf16)

            for _it in range(iters):
              pexps = {}
              qTs = {}
              kTs = {}
              psp = None  # created after v_proj's psum pool is released

              def chain(p, h, qi):
                  # one full score->softmax chain for q-tile qt = 2p+qi.
                  # edge tiles qt=0,1 only see keys in the last 128/256 band
                  # columns (the rest is zero padding) -> trim the work
                  qt = 2 * p + qi
                  lo = 256 if qt == 0 else (128 if qt == 1 else 0)
                  w = 384 - lo
                  sc_ps = psp.tile([128, 384], f32, tag="sc",
                                   name=f"sc_{p}_{h}_{qi}", bufs=2)
                  nc.tensor.matmul(
                      sc_ps[:, 0:w],
                      qTs[h][:, qt * 128:(qt + 1) * 128],
                      kTs[h][:, qt * 128 + lo: qt * 128 + 384],
                      start=True, stop=True)
                  v_idx = min(qt, 2)
                  scb = sp.tile([128, 384], bf16, tag="scb",
                                name=f"scb_{p}_{h}_{qi}")
                  nc.vector.tensor_add(out=scb[:, 0:w], in0=sc_ps[:, 0:w],
                                       in1=mb_sb[:, v_idx, lo:384])
                  pexp = pxp.tile([128, 384], bf16, tag="pexp",
                                  name=f"pexp_{p}_{h}_{qi}")
                  rs = smp.tile([128, 1], f32, tag="rs")
                  nc.scalar.activation(pexp[:, 0:w], scb[:, 0:w], AF.Exp,
                                       bias=0.0, scale=SCALE,
                                       accum_out=rs[:])
                  rr = smp.tile([128, 1], f32, tag="rr")
                  nc.vector.reciprocal(rr[:], rs[:])
                  nc.vector.tensor_scalar_mul(pexp[:, 0:w], pexp[:, 0:w],
                                              rr[:])
                  pexps[(p, h, qi)] = (pexp, lo)

              def trans_av(p, h):
                  # transpose pexp tiles for (p, h) and contract against V
                  PT = ptp.tile([128, 4, 256], bf16, tag="PT",
                                name=f"PT_{p}_{h}")
                  if p == 0:
                      # qi=0 fills cc2 block0; qi=1 fills cc2,cc3 block1
                      nc.vector.tensor_copy(PT[:, 3, 0:128], zr[:, 0:128])
                  else:
                      nc.vector.tensor_copy(PT[:, 0, 128:256], zr[:, 0:128])
                      nc.vector.tensor_copy(PT[:, 3, 0:128], zr[:, 0:128])
                  for qi in range(2):
                      pexp, lo = pexps.pop((p, h, qi))
                      nch = (384 - lo) // 128
                      pt3 = psp.tile([128, 384], bf16, tag="pt",
                                     name=f"pt3_{p}_{h}_{qi}", bufs=2)
                      for j in range(nch):
                          nc.tensor.transpose(
                              pt3[:, j * 128:(j + 1) * 128],
                              pexp[:, j * 128:(j + 1) * 128],
                              ident_sb[:])
                      # trimmed band block j is key chunk qi + lo//128 + j
                      c0 = qi + lo // 128
                      if qi == 0:
                          nc.scalar.copy(
                              PT[:, c0:c0 + nch, qi * 128:(qi + 1) * 128],
                              pt3[:, 0:nch * 128].rearrange(
                                  "p (c q) -> p c q", c=nch))
                      else:
                          nc.vector.tensor_copy(
                              PT[:, c0:c0 + nch, qi * 128:(qi + 1) * 128],
                              pt3[:, 0:nch * 128].rearrange(
                                  "p (c q) -> p c q", c=nch))
                  av = psp.tile([128, 256], f32, tag="av",
                                name=f"av_{p}_{h}", bufs=1)
                  cc0 = 2 if p == 0 else 0
                  for cc in range(cc0, 4):
                      nc.tensor.matmul(
                          av[:],
                          vP[:, 2 * p + cc, h * 128:(h + 1) * 128],
                          PT[:, cc, :],
                          start=(cc == cc0), stop=(cc == 3))
                  if h % 2 == 0:
                      nc.scalar.copy(
                          attnT[:, h, p * 256:(p + 1) * 256], av[:])
                  else:
                      nc.vector.tensor_copy(
                          attnT[:, h, p * 256:(p + 1) * 256], av[:])

              # ------------- fused qkv projection + rope + attention chains
              with ExitStack() as p1:
                xp = p1.enter_context(tc.tile_pool(name="xp", bufs=16))
                wqp = p1.enter_context(tc.tile_pool(name="wqp", bufs=3))
                tp = p1.enter_context(tc.tile_pool(name="tp", bufs=4))
                wv_stack = ExitStack()
                wvp = wv_stack.enter_context(tc.tile_pool(name="wvp", bufs=1))

                xts = []
                wv_t = wvp.tile([128, 16, 512], bf16, tag="wv")
                for c in range(16):
                    xt = xp.tile([128, S], bf16, tag="xt", name=f"xt{c}")
                    nc.sync.dma_start(wv_t[:, c, :], wvT_r[:, c, :])
                    if c == 0:
                        nc.sync.dma_start(xt[:, 0:512], xT_r[:, c, 0:512])
                        nc.sync.dma_start(xt[:, 512:], xT_r[:, c, 512:])
                    else:
                        nc.sync.dma_start(xt[:], xT_r[:, c, :])
                    xts.append(xt)
                if _it == 0:
                    nc.sync.dma_start(ident_sb[:], ident_d[:])
                    nc.sync.dma_start(cs_sb[:, 0, :], cosT_d[:])
                    nc.sync.dma_start(cs_sb[:, 1, :], sinT_d[:])
                    nc.sync.dma_start(mb_sb[:], mb_d[:])
                    for h in range(4):
                        nc.sync.dma_start(wo_sb[:, h, :], woT_r[:, h, :])

                def v_proj(vpsp):
                    # V projection: direct [s, d] orientation (lhsT = x chunk)
                    for ss in range(16):
                        psv = vpsp.tile([128, 512], f32, tag="psv",
                                        name=f"psv_{ss}")
                        for c in range(16):
                            nc.tensor.matmul(
                                psv[:], xts[c][:, ss * 128:(ss + 1) * 128],
                                wv_t[:, c, :], start=(c == 0), stop=(c == 15))
                        nc.scalar.copy(vP[:, ss + 2, :], psv[:])

                vp_stack = ExitStack()
                vpsp = vp_stack.enter_context(
                    tc.tile_pool(name="vps", bufs=8, space="PSUM"))
                v_proj(vpsp)
                wv_stack.close()

                def mt_group(mt, fillers=(), every=4, pool=None):
                    fillers = list(fillers)
                    wq_t = wqp.tile([128, S], bf16, tag="wq")
                    nc.sync.dma_start(wq_t[:], wqkP_r[:, mt, :])
                    h = mt % 4
                    is_q = mt < 4
                    if is_q:
                        qTs[h] = qkp.tile([128, S], bf16, tag="qT",
                                          name=f"qT{h}")
                    else:
                        kTs[h] = qkp.tile([128, 256 + S], bf16, tag="kT",
                                          name=f"kT{h}")
                        nc.vector.tensor_copy(kTs[h][:, 0:256], zr[:, 0:256])
                    for n in range(4):
                        if pool is None:
                            ps = psp.tile([128, 512], f32, tag="ps",
                                          name=f"ps_{mt}_{n}")
                        else:
                            ps = pool.tile([128, 512], f32, tag="psv",
                                           name=f"ps_{mt}_{n}")
                        for c in range(16):
                            nc.tensor.matmul(
                                ps[:], wq_t[:, c * 128:(c + 1) * 128],
                                xts[c][:, n * 512:(n + 1) * 512],
                                start=(c == 0), stop=(c == 15))
                            if c % every == every - 1 and fillers:
                                fillers.pop(0)()
                        tmp = tp.tile([128, 512], bf16, tag="tmp",
                                      name=f"tmp_{mt}_{n}")
                        if n % 2 == 0:
                            nc.scalar.copy(tmp[:], ps[:])
                        else:
                            nc.vector.tensor_copy(tmp[:], ps[:])
                        # rotate-half via adjacent-partition pair swap (DMA);
                        # negation folded into sinT_alt
                        sw = tp.tile([128, 512], bf16, tag="sw",
                                     name=f"sw_{mt}_{n}", bufs=3)
                        tmp_r = tmp.rearrange("(g two) s -> g two s", two=2)
                        sw_r = sw.rearrange("(g two) s -> g two s", two=2)
                        nc.sync.dma_start(sw_r[:, 0, :], tmp_r[:, 1, :])
                        nc.sync.dma_start(sw_r[:, 1, :], tmp_r[:, 0, :])
                        if is_q:
                            dest = qTs[h][:, n * 512:(n + 1) * 512]
                        else:
                            dest = kTs[h][:, 256 + n * 512: 256 + (n + 1) * 512]
                        nc.vector.tensor_mul(
                            out=dest, in0=sw[:],
                            in1=cs_sb[:, 1, n * 512:(n + 1) * 512])
                        nc.vector.tensor_mul(
                            out=tmp[:], in0=tmp[:],
                            in1=cs_sb[:, 0, n * 512:(n + 1) * 512])
                        nc.vector.tensor_add(out=dest, in0=dest, in1=tmp[:])

                # ---- staggered out-projection emission
                obs = {}

                def out_mm(st):
                    sti = st % 2
                    ob = obp.tile([128, 2048], bf16, tag="ob",
                                  name=f"ob_{st}")
                    for nt in range(4):
                        po = psp.tile([128, 512], f32, tag="ps",
                                      name=f"po_{st}_{nt}")
                        for hh in range(4):
                            nc.tensor.matmul(
                                po[:], attnT[:, hh, st * 128:(st + 1) * 128],
                                wo_sb[:, hh, nt * 512:(nt + 1) * 512],
                                start=(hh == 0), stop=(hh == 3))
                        if (sti + nt) % 2 == 0:
                            nc.scalar.copy(
                                ob[:, nt * 512:(nt + 1) * 512], po[:])
                        else:
                            nc.vector.tensor_copy(
                                ob[:, nt * 512:(nt + 1) * 512], po[:])
                        if st == 15:
                            # last tile: per-quarter part writes so the final
                            # RS only waits on a 512-col DMA
                            nc.sync.dma_start(
                                part_dram[st * 128:(st + 1) * 128,
                                          nt * 512:(nt + 1) * 512],
                                ob[:, nt * 512:(nt + 1) * 512])
                    obs[st] = ob

                def out_part(st):
                    nc.sync.dma_start(
                        part_dram[st * 128:(st + 1) * 128, :], obs.pop(st))

                def out_rs(g0, G):
                    # ReduceScatter over sts [g0, g0+G): core j of the group
                    # receives G*32 contiguous rows at input offset
                    # g0*128 + j*G*32; stored at rs rows [g0*32, (g0+G)*32)
                    if phases < 3:
                        return
                    rows = G * 32
                    rsv = rs_dram.rearrange(
                        "s (b e) -> (s b) e",
                        b=4)[g0 * 128:g0 * 128 + rows * 4]
                    if single_core:
                        # surrogate: the j=0 chunk; flat views keep it wide
                        src = part_dram.rearrange(
                            "s (b e) -> (s b) e",
                            b=4)[g0 * 512:g0 * 512 + rows * 4]
                        nc.sync.dma_start(rsv, src)
                    else:
                        nc.gpsimd.collective_compute(
                            "ReduceScatter", ALU.add, replica_groups=RG,
                            ins=[part_dram[g0 * 128:(g0 + G) * 128, :].opt()],
                            outs=[rs_dram[g0 * 32:g0 * 32 + rows, :].opt()])

                def out_final(g0, G):
                    if phases < 3:
                        return
                    rows = G * 32
                    rsv = rs_dram.rearrange(
                        "s (b e) -> (s b) e",
                        b=4)[g0 * 128:g0 * 128 + rows * 4]
                    dst = out_d.rearrange(
                        "s (b e) -> (s b) e",
                        b=4)[g0 * 128:g0 * 128 + rows * 4]
                    nc.sync.dma_start(dst, rsv)

                def out_proj_st(st):
                    # part-DMA lags the matmuls so the in-order sync-queue
                    # dispatch never waits; grouped RS/out emitted on the
                    # group boundaries (one extra st of lag)
                    out_mm(st)
                    if st >= 1:
                        out_part(st - 1)
                    if st == 7:
                        out_rs(0, 6)
                    elif st == 8:
                        out_final(0, 6)
                    elif st == 13:
                        out_rs(6, 6)
                    elif st == 14:
                        out_final(6, 6)
                    elif st == 15:
                        out_rs(12, 2)
                        out_rs(14, 1)

                def out_proj_flush():
                    obs.pop(15)
                    out_final(12, 2)
                    out_final(14, 1)
                    out_rs(15, 1)
                    out_final(15, 1)

                # ---- fused emission schedule. q0 rides the v_proj psum
                # pool (no inter-pool zone dependency); the long-lived psum
                # pool is created after that pool retires.
                for h in range(4):
                    ch = []
                    if h >= 1:
                        for p in range(8):
                            for qi in range(2):
                                ch.append(lambda p=p, qi=qi, hh=h - 1:
                                          chain(p, hh, qi))
                    if h == 0:
                        mt_group(0, pool=vpsp)                # q_0
                        vp_stack.close()
                        psp = ctx.enter_context(
                            tc.tile_pool(name="ps", bufs=3, space="PSUM"))
                        obp = p1.enter_context(
                            tc.tile_pool(name="obp", bufs=4))
                    else:
                        mt_group(h, fillers=ch, every=3)      # q_h
                    ta = []
                    if h >= 1:
                        for p in range(8):
                            ta.append(lambda p=p, hh=h - 1: trans_av(p, hh))
                    if h == 3:
                        # also squeeze the first chains(3) into k3's stream;
                        # their kT3 bands are roped by the time these slots
                        # come up (n2/n3)
                        for p in (0, 1):
                            for qi in range(2):
                                ta.append(lambda p=p, qi=qi: chain(p, 3, qi))
                        mt_group(4 + h, fillers=ta, every=3)  # k_3
                    else:
                        mt_group(4 + h, fillers=ta, every=8)  # k_h

                # tail: chains(3) / trans_av(3) / out-projection pipelined
                for p in range(2, 8):
                    chain(p, 3, 0)
                    chain(p, 3, 1)
                    trans_av(p - 2, 3)
                    if p >= 3:
                        pp = p - 3
                        out_proj_st(2 * pp)
                        out_proj_st(2 * pp + 1)
                trans_av(6, 3)
                out_proj_st(10)
                out_proj_st(11)
                trans_av(7, 3)
                for st in (12, 13, 14, 15):
                    out_proj_st(st)
                out_proj_flush()

              if phases < 2:
                  ob1 = resid.tile([128, 512], f32, tag="ob1")
                  nc.vector.tensor_copy(ob1[:], qTs[3][:, 0:512])
                  nc.sync.dma_start(out_d[0:128, 0:512], ob1[:])
                  continue

    nc.compile()
    return nc


def _get_nc():
    if "nc" not in _CACHE:
        _CACHE["nc"] = _build_nc()
    return _CACHE["nc"]


# ----------------------------------------------------------------- entry
def _prepare_in_maps(x, w_qkv, w_out, w_c1, w_c2):
    import ml_dtypes

    x = np.asarray(x, dtype=np.float32)
    w_qkv = np.asarray(w_qkv, dtype=np.float32)
    w_out = np.asarray(w_out, dtype=np.float32)
    w_c1 = np.asarray(w_c1, dtype=np.float32)
    w_c2 = np.asarray(w_c2, dtype=np.float32)

    window = _host_window(x, w_c1, w_c2)
    mb = _build_maskbias(window)
    cosT, sinT = _rope_tables()
    ident = np.eye(128, dtype=np.float32)
    bf = ml_dtypes.bfloat16

    in_maps = []
    for c in range(NCORES):
        b, g = divmod(c, 4)
        rows = slice(g * 512, g * 512 + 512)
        wq = w_qkv[0 * EMB:1 * EMB][rows]
        wk = w_qkv[1 * EMB:2 * EMB][rows]
        wv = w_qkv[2 * EMB:3 * EMB][rows]
        # packed qk weights: wqkP[mt, p, c*128 + j] = wqk[e = c*128+p, mt*128+j]
        wqkT = np.concatenate([wq, wk], 0).T  # [EMB, 1024]
        wqkP = np.ascontiguousarray(
            wqkT.reshape(16, 128, 8, 128).transpose(2, 1, 0, 3)
        ).reshape(8 * 128, 2048)
        in_maps.append({
            "xT": np.ascontiguousarray(x[b].T).astype(bf),
            "wqkP": wqkP.astype(bf),
            "wvT": np.ascontiguousarray(wv.T).astype(bf),
            "woT": np.ascontiguousarray(w_out.T[rows]).astype(bf),
            "cosT": cosT.astype(bf), "sinT": sinT.astype(bf),
            "ident": ident.astype(bf),
            "mb": mb,
        })
    return in_maps


def _assemble(results):
    out = np.empty((B, S, EMB), dtype=np.float32)
    for c in range(NCORES):
        b, j = divmod(c, 4)
        r = np.asarray(results[c]["out"], dtype=np.float32)  # [512, EMB] bf16
        for g0, G in ((0, 6), (6, 6), (12, 2), (14, 1), (15, 1)):
            rows = G * 32
            out[b, g0 * 128 + j * rows: g0 * 128 + (j + 1) * rows] = \
                r[g0 * 32:g0 * 32 + rows]
    return out


def kernel(x, w_qkv, w_out, w_c1, w_c2):
    from concourse.bass_utils import run_bass_kernel_spmd

    nc = _get_nc()
    in_maps = _prepare_in_maps(x, w_qkv, w_out, w_c1, w_c2)
    res = run_bass_kernel_spmd(nc, in_maps, core_ids=list(range(NCORES)))
    return _assemble(res.results)


# revision 4
# speedup vs baseline: 1.0074x; 1.0044x over previous
"""AdaptiveWindowAttention distributed Bass kernel for 8 TRN2 NeuronCores.

Sharding: core c -> batch b = c//4, head group g = c%4 (heads 4g..4g+3).
 - QKV projection: bf16 matmuls (x, weights shipped pre-transposed+bf16),
   f32 PSUM accumulation. wq shipped host-packed so each mt group is one
   contiguous 4KB-per-partition DMA.
 - RoPE rotate-half via SBUF->SBUF DMA pair-swap of adjacent partitions
   (negation folded into the host-built sin table) + DVE combine -> bf16 q/k.
 - Windowed causal attention: window <= 256 -> 384-wide band per q-tile
   against 256-zero-padded K/V (edge q-tiles trimmed). Mask via DVE add of
   host-built bias tables; exp+rowsum on Act; normalize on DVE; probs bf16.
 - P transposed on PE (bf16, 1 cyc/row), AV + out-projection feed from
   SBUF-resident bf16 tiles. Fully fused pipeline: score/softmax chains and
   transpose+AV are emitted interleaved inside the projection matmul streams
   so no engine queue head-of-line blocks.
 - Out-projection partials written bf16 per 128-row tile; per-tile
   ReduceScatter (add) across each 4-core group into an internal DRAM
   buffer, then copied to the bf16 output. All output-path DMA dispatches
   are lag-staggered so the in-order sync queue never waits. Host casts to
   f32 and reassembles the full [2,2048,2048] output.
"""
import os
import sys

import numpy as np

for _p in ("/opt/trn_rl_repo",):
    if _p not in sys.path and os.path.isdir(_p):
        sys.path.insert(0, _p)

EMB = 2048
HEADS = 16
HD = 128
S = 2048
B = 2
SCALE = HD ** -0.5
NCORES = 8

_CACHE = {}


# ----------------------------------------------------------------- host math
def _host_window(x, w_c1, w_c2):
    xf = x.reshape(B, -1).astype(np.float64)
    var = xf.var(axis=1, ddof=1)
    var_norm = 1.0 / (1.0 + np.exp(-(var * 10.0 - 5.0)))
    x_mean = x.mean(axis=1).astype(np.float64)
    h = x_mean @ w_c1.T.astype(np.float64)
    h = h / (1.0 + np.exp(-h))
    learned = 1.0 / (1.0 + np.exp(-(h @ w_c2.T.astype(np.float64))))[:, 0]
    complexity = (var_norm + learned) / 2.0
    window_f = 64.0 + complexity * (256.0 - 64.0)
    w = int(np.float32(window_f.mean()))
    return max(min(w, S), 64)


def _build_maskbias(window):
    NEG = -1.0e5
    mb = np.empty((128, 3, 384), dtype=np.float32)
    jmin = [256, 128, 0]
    q = np.arange(128)[:, None]
    j = np.arange(384)[None, :]
    rel = q + 256 - j
    for v in range(3):
        keep = (rel >= 0) & (rel < window) & (j >= jmin[v])
        mb[:, v, :] = np.where(keep, 0.0, NEG)
    return mb


def _rope_tables():
    inv_freq = 1.0 / (10000.0 ** (np.arange(0, HD, 2, dtype=np.float32) / HD))
    pos = np.arange(S, dtype=np.float32)
    freqs = np.outer(pos, inv_freq)
    emb = np.concatenate([freqs, freqs], axis=-1)  # [S, 128]
    cosT = np.ascontiguousarray(np.cos(emb).T).astype(np.float32)
    sinT = np.ascontiguousarray(np.sin(emb).T).astype(np.float32)
    # negation of rotate-half folded into the sin table: even rows flip sign
    # (dest = pairswap(q) * sinT_alt, pairswap has no negation)
    sinT_alt = sinT.copy()
    sinT_alt[0::2, :] *= -1.0
    return cosT, sinT_alt


# ----------------------------------------------------------------- bass build
def _build_nc(single_core=False, phases=3, iters=1):
    import concourse.bass as bass  # noqa: F401
    from concourse import bacc, mybir, tile

    f32 = mybir.dt.float32
    bf16 = mybir.dt.bfloat16
    AF = mybir.ActivationFunctionType
    ALU = mybir.AluOpType

    nc = bacc.Bacc("TRN2", target_bir_lowering=False, debug=False,
                   num_devices=1 if single_core else NCORES)

    xT_d = nc.dram_tensor("xT", [EMB, S], bf16, kind="ExternalInput").ap()
    wqkP_d = nc.dram_tensor("wqkP", [8 * 128, S], bf16, kind="ExternalInput").ap()
    wvT_d = nc.dram_tensor("wvT", [EMB, 512], bf16, kind="ExternalInput").ap()
    woT_d = nc.dram_tensor("woT", [512, EMB], bf16, kind="ExternalInput").ap()
    cosT_d = nc.dram_tensor("cosT", [HD, S], bf16, kind="ExternalInput").ap()
    sinT_d = nc.dram_tensor("sinT", [HD, S], bf16, kind="ExternalInput").ap()
    ident_d = nc.dram_tensor("ident", [HD, HD], bf16, kind="ExternalInput").ap()
    mb_d = nc.dram_tensor("mb", [128, 3, 384], f32, kind="ExternalInput").ap()
    out_d = nc.dram_tensor("out", [512, EMB], bf16, kind="ExternalOutput").ap()

    xT_r = xT_d.rearrange("(c p) s -> p c s", p=128)        # [128,16,S]
    wqkP_r = wqkP_d.rearrange("(m p) n -> p m n", p=128)    # [128,8,2048]
    wvT_r = wvT_d.rearrange("(c p) m -> p c m", p=128)      # [128,16,512]
    woT_r = woT_d.rearrange("(h p) n -> p h n", p=128)      # [128,4,EMB]

    RG = [[0, 1, 2, 3], [4, 5, 6, 7]]

    with tile.TileContext(nc) as tc:
        from contextlib import ExitStack
        with ExitStack() as ctx:
            resid = ctx.enter_context(tc.tile_pool(name="resid", bufs=1))
            dramp = ctx.enter_context(tc.tile_pool(name="dram", bufs=1, space="DRAM"))
            sp = ctx.enter_context(tc.tile_pool(name="sp", bufs=4))
            pxp = ctx.enter_context(tc.tile_pool(name="pxp", bufs=18))
            smp = ctx.enter_context(tc.tile_pool(name="smp", bufs=24))
            ptp = ctx.enter_context(tc.tile_pool(name="ptp", bufs=4))
            qkp = ctx.enter_context(tc.tile_pool(name="qkp", bufs=2))

            vP = resid.tile([128, 18, 512], bf16, tag="vP")
            ident_sb = resid.tile([128, 128], bf16, tag="ident")
            cs_sb = resid.tile([128, 2, S], bf16, tag="cs")
            mb_sb = resid.tile([128, 3, 384], f32, tag="mb")
            wo_sb = resid.tile([128, 4, EMB], bf16, tag="wo")
            attnT = resid.tile([128, 4, S], bf16, tag="attnT")
            zr = resid.tile([128, 512], bf16, tag="zr")

            nc.vector.memset(zr[:], 0.0)
            # warm-up matmuls on zeros: start the PE p-state ramp during the
            # initial input-DMA window so real matmuls run at full clock
            warm_stack = ExitStack()
            warmp = warm_stack.enter_context(
                tc.tile_pool(name="warm", bufs=1, space="PSUM"))
            wps = warmp.tile([128, 512], f32, tag="w")
            for _w in range(7):
                nc.tensor.matmul(wps[:], zr[:, 0:128], zr[:],
                                 start=True, stop=True)
            warm_stack.close()
            for _c in range(2):
                nc.vector.tensor_copy(vP[:, _c, :], zr[:])

            part_dram = dramp.tile([S, EMB], bf16)
            rs_dram = dramp.tile([512, EMB], bf16)

            for _it in range(iters):
              pexps = {}
              qTs = {}
              kTs = {}
              psp = None  # created after v_proj's psum pool is released

              def chain(p, h, qi):
                  # one full score->softmax chain for q-tile qt = 2p+qi.
                  # edge tiles qt=0,1 only see keys in the last 128/256 band
                  # columns (the rest is zero padding) -> trim the work
                  qt = 2 * p + qi
                  lo = 256 if qt == 0 else (128 if qt == 1 else 0)
                  w = 384 - lo
                  sc_ps = psp.tile([128, 384], f32, tag="sc",
                                   name=f"sc_{p}_{h}_{qi}", bufs=2)
                  nc.tensor.matmul(
                      sc_ps[:, 0:w],
                      qTs[h][:, qt * 128:(qt + 1) * 128],
                      kTs[h][:, qt * 128 + lo: qt * 128 + 384],
                      start=True, stop=True)
                  v_idx = min(qt, 2)
                  scb = sp.tile([128, 384], bf16, tag="scb",
                                name=f"scb_{p}_{h}_{qi}")
                  nc.vector.tensor_add(out=scb[:, 0:w], in0=sc_ps[:, 0:w],
                                       in1=mb_sb[:, v_idx, lo:384])
                  pexp = pxp.tile([128, 384], bf16, tag="pexp",
                                  name=f"pexp_{p}_{h}_{qi}")
                  rs = smp.tile([128, 1], f32, tag="rs")
                  nc.scalar.activation(pexp[:, 0:w], scb[:, 0:w], AF.Exp,
                                       bias=0.0, scale=SCALE,
                                       accum_out=rs[:])
                  rr = smp.tile([128, 1], f32, tag="rr")
                  nc.vector.reciprocal(rr[:], rs[:])
                  nc.vector.tensor_scalar_mul(pexp[:, 0:w], pexp[:, 0:w],
                                              rr[:])
                  pexps[(p, h, qi)] = (pexp, lo)

              def trans_av(p, h):
                  # transpose pexp tiles for (p, h) and contract against V
                  PT = ptp.tile([128, 4, 256], bf16, tag="PT",
                                name=f"PT_{p}_{h}")
                  if p == 0:
                      # qi=0 fills cc2 block0; qi=1 fills cc2,cc3 block1
                      nc.vector.tensor_copy(PT[:, 3, 0:128], zr[:, 0:128])
                  else:
                      nc.vector.tensor_copy(PT[:, 0, 128:256], zr[:, 0:128])
                      nc.vector.tensor_copy(PT[:, 3, 0:128], zr[:, 0:128])
                  for qi in range(2):
                      pexp, lo = pexps.pop((p, h, qi))
                      nch = (384 - lo) // 128
                      pt3 = psp.tile([128, 384], bf16, tag="pt",
                                     name=f"pt3_{p}_{h}_{qi}", bufs=2)
                      for j in range(nch):
                          nc.tensor.transpose(
                              pt3[:, j * 128:(j + 1) * 128],
                              pexp[:, j * 128:(j + 1) * 128],
                              ident_sb[:])
                      # trimmed band block j is key chunk qi + lo//128 + j
                      c0 = qi + lo // 128
                      if qi == 0:
                          nc.scalar.copy(
                              PT[:, c0:c0 + nch, qi * 128:(qi + 1) * 128],
                              pt3[:, 0:nch * 128].rearrange(
                                  "p (c q) -> p c q", c=nch))
                      else:
                          nc.vector.tensor_copy(
                              PT[:, c0:c0 + nch, qi * 128:(qi + 1) * 128],
                              pt3[:, 0:nch * 128].rearrange(
                                  "p (c q) -> p c q", c=nch))
                  av = psp.tile([128, 256], f32, tag="av",
                                name=f"av_{p}_{h}", bufs=1)
                  cc0 = 2 if p == 0 else 0
                  for cc in range(cc0, 4):
                      nc.tensor.matmul(
                          av[:],
                          vP[:, 2 * p + cc, h * 128:(h + 1) * 128],
                          PT[:, cc, :],
                          start=(cc == cc0), stop=(cc == 3))
                  if h % 2 == 0:
                      nc.scalar.copy(
                          attnT[:, h, p * 256:(p + 1) * 256], av[:])
                  else:
                      nc.vector.tensor_copy(
                          attnT[:, h, p * 256:(p + 1) * 256], av[:])

              # ------------- fused qkv projection + rope + attention chains
              with ExitStack() as p1:
                xp = p1.enter_context(tc.tile_pool(name="xp", bufs=16))
                wqp = p1.enter_context(tc.tile_pool(name="wqp", bufs=3))
                tp = p1.enter_context(tc.tile_pool(name="tp", bufs=4))
                wv_stack = ExitStack()
                wvp = wv_stack.enter_context(tc.tile_pool(name="wvp", bufs=1))

                xts = []
                wv_t = wvp.tile([128, 16, 512], bf16, tag="wv")
                for c in range(16):
                    xt = xp.tile([128, S], bf16, tag="xt", name=f"xt{c}")
                    nc.sync.dma_start(wv_t[:, c, :], wvT_r[:, c, :])
                    if c == 0:
                        nc.sync.dma_start(xt[:, 0:512], xT_r[:, c, 0:512])
                        nc.sync.dma_start(xt[:, 512:], xT_r[:, c, 512:])
                    else:
                        nc.sync.dma_start(xt[:], xT_r[:, c, :])
                    xts.append(xt)
                if _it == 0:
                    nc.sync.dma_start(ident_sb[:], ident_d[:])
                    nc.sync.dma_start(cs_sb[:, 0, :], cosT_d[:])
                    nc.sync.dma_start(cs_sb[:, 1, :], sinT_d[:])
                    nc.sync.dma_start(mb_sb[:], mb_d[:])
                    for h in range(4):
                        nc.sync.dma_start(wo_sb[:, h, :], woT_r[:, h, :])

                def v_proj(vpsp):
                    # V projection: direct [s, d] orientation (lhsT = x chunk)
                    for ss in range(16):
                        psv = vpsp.tile([128, 512], f32, tag="psv",
                                        name=f"psv_{ss}")
                        for c in range(16):
                            nc.tensor.matmul(
                                psv[:], xts[c][:, ss * 128:(ss + 1) * 128],
                                wv_t[:, c, :], start=(c == 0), stop=(c == 15))
                        nc.scalar.copy(vP[:, ss + 2, :], psv[:])

                vp_stack = ExitStack()
                vpsp = vp_stack.enter_context(
                    tc.tile_pool(name="vps", bufs=8, space="PSUM"))
                v_proj(vpsp)
                wv_stack.close()

                def mt_group(mt, fillers=(), every=4, pool=None):
                    fillers = list(fillers)
                    wq_t = wqp.tile([128, S], bf16, tag="wq")
                    nc.sync.dma_start(wq_t[:], wqkP_r[:, mt, :])
                    h = mt % 4
                    is_q = mt < 4
                    if is_q:
                        qTs[h] = qkp.tile([128, S], bf16, tag="qT",
                                          name=f"qT{h}")
                    else:
                        kTs[h] = qkp.tile([128, 256 + S], bf16, tag="kT",
                                          name=f"kT{h}")
                        nc.vector.tensor_copy(kTs[h][:, 0:256], zr[:, 0:256])
                    for n in range(4):
                        if pool is None:
                            ps = psp.tile([128, 512], f32, tag="ps",
                                          name=f"ps_{mt}_{n}")
                        else:
                            ps = pool.tile([128, 512], f32, tag="psv",
                                           name=f"ps_{mt}_{n}")
                        for c in range(16):
                            nc.tensor.matmul(
                                ps[:], wq_t[:, c * 128:(c + 1) * 128],
                                xts[c][:, n * 512:(n + 1) * 512],
                                start=(c == 0), stop=(c == 15))
                            if c % every == every - 1 and fillers:
                                fillers.pop(0)()
                        tmp = tp.tile([128, 512], bf16, tag="tmp",
                                      name=f"tmp_{mt}_{n}")
                        nc.scalar.copy(tmp[:], ps[:])
                        # rotate-half via adjacent-partition pair swap (DMA,
                        # dispatched from the Act HWDGE queue right after the
                        # Act drain so neither SP nor Act ever waits inline);
                        # negation folded into sinT_alt
                        sw = tp.tile([128, 512], bf16, tag="sw",
                                     name=f"sw_{mt}_{n}", bufs=3)
                        tmp_r = tmp.rearrange("(g two) s -> g two s", two=2)
                        sw_r = sw.rearrange("(g two) s -> g two s", two=2)
                        nc.scalar.dma_start(sw_r[:, 0, :], tmp_r[:, 1, :])
                        nc.scalar.dma_start(sw_r[:, 1, :], tmp_r[:, 0, :])
                        if is_q:
                            dest = qTs[h][:, n * 512:(n + 1) * 512]
                        else:
                            dest = kTs[h][:, 256 + n * 512: 256 + (n + 1) * 512]
                        nc.vector.tensor_mul(
                            out=dest, in0=sw[:],
                            in1=cs_sb[:, 1, n * 512:(n + 1) * 512])
                        nc.vector.tensor_mul(
                            out=tmp[:], in0=tmp[:],
                            in1=cs_sb[:, 0, n * 512:(n + 1) * 512])
                        nc.vector.tensor_add(out=dest, in0=dest, in1=tmp[:])

                # ---- staggered out-projection emission
                obs = {}

                def out_mm(st):
                    sti = st % 2
                    ob = obp.tile([128, 2048], bf16, tag="ob",
                                  name=f"ob_{st}")
                    for nt in range(4):
                        po = psp.tile([128, 512], f32, tag="ps",
                                      name=f"po_{st}_{nt}")
                        for hh in range(4):
                            nc.tensor.matmul(
                                po[:], attnT[:, hh, st * 128:(st + 1) * 128],
                                wo_sb[:, hh, nt * 512:(nt + 1) * 512],
                                start=(hh == 0), stop=(hh == 3))
                        if (sti + nt) % 2 == 0:
                            nc.scalar.copy(
                                ob[:, nt * 512:(nt + 1) * 512], po[:])
                        else:
                            nc.vector.tensor_copy(
                                ob[:, nt * 512:(nt + 1) * 512], po[:])
                        if st == 15:
                            # last tile: per-quarter part writes so the final
                            # RS only waits on a 512-col DMA
                            nc.sync.dma_start(
                                part_dram[st * 128:(st + 1) * 128,
                                          nt * 512:(nt + 1) * 512],
                                ob[:, nt * 512:(nt + 1) * 512])
                    obs[st] = ob

                def out_part(st):
                    nc.sync.dma_start(
                        part_dram[st * 128:(st + 1) * 128, :], obs.pop(st))

                def out_rs(g0, G):
                    # ReduceScatter over sts [g0, g0+G): core j of the group
                    # receives G*32 contiguous rows at input offset
                    # g0*128 + j*G*32; stored at rs rows [g0*32, (g0+G)*32)
                    if phases < 3:
                        return
                    rows = G * 32
                    rsv = rs_dram.rearrange(
                        "s (b e) -> (s b) e",
                        b=4)[g0 * 128:g0 * 128 + rows * 4]
                    if single_core:
                        # surrogate: the j=0 chunk; flat views keep it wide
                        src = part_dram.rearrange(
                            "s (b e) -> (s b) e",
                            b=4)[g0 * 512:g0 * 512 + rows * 4]
                        nc.sync.dma_start(rsv, src)
                    else:
                        nc.gpsimd.collective_compute(
                            "ReduceScatter", ALU.add, replica_groups=RG,
                            ins=[part_dram[g0 * 128:(g0 + G) * 128, :].opt()],
                            outs=[rs_dram[g0 * 32:g0 * 32 + rows, :].opt()])

                def out_final(g0, G):
                    if phases < 3:
                        return
                    rows = G * 32
                    rsv = rs_dram.rearrange(
                        "s (b e) -> (s b) e",
                        b=4)[g0 * 128:g0 * 128 + rows * 4]
                    dst = out_d.rearrange(
                        "s (b e) -> (s b) e",
                        b=4)[g0 * 128:g0 * 128 + rows * 4]
                    nc.sync.dma_start(dst, rsv)

                def out_proj_st(st):
                    # part-DMA lags the matmuls so the in-order sync-queue
                    # dispatch never waits; grouped RS/out emitted on the
                    # group boundaries (one extra st of lag)
                    out_mm(st)
                    if st >= 1:
                        out_part(st - 1)
                    if st == 7:
                        out_rs(0, 6)
                    elif st == 8:
                        out_final(0, 6)
                    elif st == 13:
                        out_rs(6, 6)
                    elif st == 14:
                        out_final(6, 6)
                    elif st == 15:
                        out_rs(12, 2)
                        out_rs(14, 1)

                def out_proj_flush():
                    obs.pop(15)
                    out_final(12, 2)
                    out_final(14, 1)
                    out_rs(15, 1)
                    out_final(15, 1)

                # ---- fused emission schedule. q0 rides the v_proj psum
                # pool (no inter-pool zone dependency); the long-lived psum
                # pool is created after that pool retires.
                for h in range(4):
                    ch = []
                    if h >= 1:
                        for p in range(8):
                            for qi in range(2):
                                ch.append(lambda p=p, qi=qi, hh=h - 1:
                                          chain(p, hh, qi))
                    if h == 0:
                        mt_group(0, pool=vpsp)                # q_0
                        vp_stack.close()
                        psp = ctx.enter_context(
                            tc.tile_pool(name="ps", bufs=3, space="PSUM"))
                        obp = p1.enter_context(
                            tc.tile_pool(name="obp", bufs=4))
                    else:
                        mt_group(h, fillers=ch, every=3)      # q_h
                    ta = []
                    if h >= 1:
                        for p in range(8):
                            ta.append(lambda p=p, hh=h - 1: trans_av(p, hh))
                    if h == 3:
                        # also squeeze the first chains(3) into k3's stream;
                        # their kT3 bands are roped by the time these slots
                        # come up (n2/n3)
                        for p in (0, 1):
                            for qi in range(2):
                                ta.append(lambda p=p, qi=qi: chain(p, 3, qi))
                        mt_group(4 + h, fillers=ta, every=3)  # k_3
                    else:
                        mt_group(4 + h, fillers=ta, every=8)  # k_h

                # tail: chains(3) / trans_av(3) / out-projection pipelined
                for p in range(2, 8):
                    chain(p, 3, 0)
                    chain(p, 3, 1)
                    trans_av(p - 2, 3)
                    if p >= 3:
                        pp = p - 3
                        out_proj_st(2 * pp)
                        out_proj_st(2 * pp + 1)
                trans_av(6, 3)
                out_proj_st(10)
                out_proj_st(11)
                trans_av(7, 3)
                for st in (12, 13, 14, 15):
                    out_proj_st(st)
                out_proj_flush()

              if phases < 2:
                  ob1 = resid.tile([128, 512], f32, tag="ob1")
                  nc.vector.tensor_copy(ob1[:], qTs[3][:, 0:512])
                  nc.sync.dma_start(out_d[0:128, 0:512], ob1[:])
                  continue

    nc.compile()
    return nc


def _get_nc():
    if "nc" not in _CACHE:
        _CACHE["nc"] = _build_nc()
    return _CACHE["nc"]


# ----------------------------------------------------------------- entry
def _prepare_in_maps(x, w_qkv, w_out, w_c1, w_c2):
    import ml_dtypes

    x = np.asarray(x, dtype=np.float32)
    w_qkv = np.asarray(w_qkv, dtype=np.float32)
    w_out = np.asarray(w_out, dtype=np.float32)
    w_c1 = np.asarray(w_c1, dtype=np.float32)
    w_c2 = np.asarray(w_c2, dtype=np.float32)

    window = _host_window(x, w_c1, w_c2)
    mb = _build_maskbias(window)
    cosT, sinT = _rope_tables()
    ident = np.eye(128, dtype=np.float32)
    bf = ml_dtypes.bfloat16

    in_maps = []
    for c in range(NCORES):
        b, g = divmod(c, 4)
        rows = slice(g * 512, g * 512 + 512)
        wq = w_qkv[0 * EMB:1 * EMB][rows]
        wk = w_qkv[1 * EMB:2 * EMB][rows]
        wv = w_qkv[2 * EMB:3 * EMB][rows]
        # packed qk weights: wqkP[mt, p, c*128 + j] = wqk[e = c*128+p, mt*128+j]
        wqkT = np.concatenate([wq, wk], 0).T  # [EMB, 1024]
        wqkP = np.ascontiguousarray(
            wqkT.reshape(16, 128, 8, 128).transpose(2, 1, 0, 3)
        ).reshape(8 * 128, 2048)
        in_maps.append({
            "xT": np.ascontiguousarray(x[b].T).astype(bf),
            "wqkP": wqkP.astype(bf),
            "wvT": np.ascontiguousarray(wv.T).astype(bf),
            "woT": np.ascontiguousarray(w_out.T[rows]).astype(bf),
            "cosT": cosT.astype(bf), "sinT": sinT.astype(bf),
            "ident": ident.astype(bf),
            "mb": mb,
        })
    return in_maps


def _assemble(results):
    out = np.empty((B, S, EMB), dtype=np.float32)
    for c in range(NCORES):
        b, j = divmod(c, 4)
        r = np.asarray(results[c]["out"], dtype=np.float32)  # [512, EMB] bf16
        for g0, G in ((0, 6), (6, 6), (12, 2), (14, 1), (15, 1)):
            rows = G * 32
            out[b, g0 * 128 + j * rows: g0 * 128 + (j + 1) * rows] = \
                r[g0 * 32:g0 * 32 + rows]
    return out


def kernel(x, w_qkv, w_out, w_c1, w_c2):
    from concourse.bass_utils import run_bass_kernel_spmd

    nc = _get_nc()
    in_maps = _prepare_in_maps(x, w_qkv, w_out, w_c1, w_c2)
    res = run_bass_kernel_spmd(nc, in_maps, core_ids=list(range(NCORES)))
    return _assemble(res.results)
